# revision 7
# baseline (speedup 1.0000x reference)
"""Trainium2 Bass kernel for nn_LocallyConnectedGC.

out[b, m, f] = sum_n x[b, n, f] * (support * kernel)[n, m] + bias[f]

Strategy: data-parallel over batch across 8 NeuronCores (32 batches/core).
Per core: W = support*kernel computed once on DVE; per batch a dense
[199,199]^T @ [199,1024] matmul on TensorE (float32r operands, fp32 PSUM
accumulation over the two K tiles), bias fused into the PSUM->SBUF eviction.
"""

import sys
from contextlib import ExitStack

sys.path.insert(0, "/opt/trn_rl_repo")

import numpy as np

import concourse.bass as bass  # noqa: F401  (engine types)
import concourse.tile as tile
from concourse import bacc, mybir
from concourse.bass_utils import run_bass_kernel_spmd

N_CORES = 8
B_FULL, N, F = 256, 199, 1024
B_PER = B_FULL // N_CORES  # 32
K1 = 128  # first K/M tile size
K2 = N - K1  # 71
NCHUNK = 512  # fp32 matmul moving-operand max / one PSUM bank

F32 = mybir.dt.float32
F32R = mybir.dt.float32r


HALO = 3  # K_HOP: support mask is zero outside |n-m| <= 3 (mod N)
MA = 128 - 2 * HALO  # 122 output rows for tile A
MB = N - MA  # 77 output rows for tile B
KB = MB + 2 * HALO  # 83 contraction rows for tile B


def build_tile_kernel(tc, x_ap, sup_ap, ker_ap, bias_ap, out_ap, b_per, mm_dtype=F32):
    """Banded single-pass formulation.

    W = support*kernel is banded (7 diagonals, wrap-around). Output rows
    m in [0, MA) only consume x rows n in {N-3..N-1, 0..MA+2} -- exactly 128
    rows -- and rows m in [MA, N) consume n in {MA-3..N-1, 0..2} -- KB rows.
    So each output tile is ONE matmul over a wrapped K-window instead of a
    2-pass K accumulation: half the PE work, exact fp32.
    """
    nc = tc.nc
    ctx = ExitStack()

    wpool = ctx.enter_context(tc.tile_pool(name="w", bufs=1))
    xpool = ctx.enter_context(tc.tile_pool(name="x", bufs=4))
    opool = ctx.enter_context(tc.tile_pool(name="o", bufs=4))
    ppool = ctx.enter_context(tc.tile_pool(name="p", bufs=4, space="PSUM"))

    # Weight tiles in the same wrapped-window partition layout as the x tiles.
    # Window A partitions: [N-3..N-1] ++ [0..125);  window B: [119..N) ++ [0..3)
    def load_windowed(pool, src_ap, tag, which):
        if which == "A":
            t = pool.tile([128, N], F32, tag=tag)
            nc.sync.dma_start(t[0:HALO, :], src_ap[N - HALO : N, :])
            nc.sync.dma_start(t[HALO:128, :], src_ap[0 : 128 - HALO, :])
        else:
            t = pool.tile([KB, N], F32, tag=tag)
            nc.sync.dma_start(t[0 : KB - HALO, :], src_ap[MA - HALO : N, :])
            nc.sync.dma_start(t[KB - HALO : KB, :], src_ap[0:HALO, :])
        return t

    sA = load_windowed(wpool, sup_ap, "sA", "A")
    kA = load_windowed(wpool, ker_ap, "kA", "A")
    wA = wpool.tile([128, N], F32, tag="wA")
    nc.vector.tensor_mul(wA[:], sA[:], kA[:])

    sB = load_windowed(wpool, sup_ap, "sB", "B")
    kB_ = load_windowed(wpool, ker_ap, "kB", "B")
    wB = wpool.tile([KB, N], F32, tag="wB")
    nc.vector.tensor_mul(wB[:], sB[:], kB_[:])

    # bias broadcast to all 128 partitions (stride-0 partition read from DRAM)
    bb = wpool.tile([128, F], F32, tag="bb")
    nc.sync.dma_start(bb[:], bias_ap.partition_broadcast(128))

    for b in range(b_per):
        xA = xpool.tile([128, F], F32, tag="xA")
        nc.sync.dma_start(xA[0:HALO, :], x_ap[b, N - HALO : N, :])
        nc.sync.dma_start(xA[HALO:128, :], x_ap[b, 0 : 128 - HALO, :])
        xB = xpool.tile([KB, F], F32, tag="xB")
        nc.sync.dma_start(xB[0 : KB - HALO, :], x_ap[b, MA - HALO : N, :])
        nc.sync.dma_start(xB[KB - HALO : KB, :], x_ap[b, 0:HALO, :])

        for (w, xt, m0, mP) in ((wA, xA, 0, MA), (wB, xB, MA, MB)):
            ps = ppool.tile([128, F], F32, tag="ps")
            for nch in range(0, F, NCHUNK):
                nc.tensor.matmul(
                    ps[0:mP, nch : nch + NCHUNK],
                    w[:, m0 : m0 + mP],
                    xt[:, nch : nch + NCHUNK],
                    start=True,
                    stop=True,
                )
            ot = opool.tile([128, F], F32, tag="ot")
            nc.vector.tensor_add(ot[0:mP, :], ps[0:mP, :], bb[0:mP, :])
            nc.sync.dma_start(out_ap[b, m0 : m0 + mP, :], ot[0:mP, :])

    ctx.close()


def build_nc(b_per=B_PER, mm_dtype=F32):
    nc = bacc.Bacc("TRN2", target_bir_lowering=False, debug=False)
    x_d = nc.dram_tensor("x", [b_per, N, F], F32, kind="ExternalInput")
    sup_d = nc.dram_tensor("support", [N, N], F32, kind="ExternalInput")
    ker_d = nc.dram_tensor("kernel", [N, N], F32, kind="ExternalInput")
    bias_d = nc.dram_tensor("bias", [F], F32, kind="ExternalInput")
    out_d = nc.dram_tensor("out", [b_per, N, F], F32, kind="ExternalOutput")

    with tile.TileContext(nc) as tc:
        build_tile_kernel(
            tc, x_d.ap(), sup_d.ap(), ker_d.ap(), bias_d.ap(), out_d.ap(), b_per,
            mm_dtype=mm_dtype,
        )
    nc.compile()
    return nc


_NC_CACHE = {}


def kernel(x, support, kernel, bias):
    if "nc" not in _NC_CACHE:
        _NC_CACHE["nc"] = build_nc()
    nc = _NC_CACHE["nc"]
    x = np.ascontiguousarray(x, dtype=np.float32)
    support = np.ascontiguousarray(support, dtype=np.float32)
    kernel = np.ascontiguousarray(kernel, dtype=np.float32)
    bias = np.ascontiguousarray(bias, dtype=np.float32)
    in_maps = [
        {
            "x": x[i * B_PER : (i + 1) * B_PER],
            "support": support,
            "kernel": kernel,
            "bias": bias,
        }
        for i in range(N_CORES)
    ]
    res = run_bass_kernel_spmd(nc, in_maps, core_ids=list(range(N_CORES)))
    return np.concatenate([r["out"] for r in res.results], axis=0)
